# revision 4
# baseline (speedup 1.0000x reference)
"""Trainium2 Bass kernel for the DSIB InfoNCE loss (fp8 DoubleRow).

Data-parallel over the 512 y rows: 8 cores x 64 rows; each core computes
its [64, 512] score block, reduces it to per-row (logsumexp, diag) pairs,
and the host sums the partials (the all-reduce of the sharding hint).

Per-core pipeline (software-pipelined slots: h1(t+1) | L2(t) | relu(t-1)
| L3(t-2), so no engine waits on a same-slot cross-engine dependency):
  * L1 linearity: A = X @ W1x is precomputed once ([256, 512] f16,
    hid-in on partitions); c_i = Y_i @ W1y + b1 per row.  DVE writes
    h1 = relu(A + c_i) straight into a [128, 1024] fp8(e4m3) tile (the
    two 128-channel hid-in chunks side by side = the k-major DoubleRow
    moving layout).
  * L2 runs as fp8 DoubleRow matmuls (K=256 contracted in one pass via
    [128, 2, *] operand views).  W2 ships as an fp8 hi+lo pair at a
    common x16 scale accumulating in one PSUM tile: W2's quantization
    error dominates the fp8 loss error, while h1's own fp8 error largely
    cancels between the diag and lse means, so compensating W2 alone
    holds the final rel err at ~4e-3 (vs ~2e-2 uncompensated).
  * h2 = relu(z16 + 16*b2) -> f16 (values x16; the global 1/16 is folded
    into the L3 weights).  One ACT instruction for hid-out block 0, one
    DVE instruction for block 1 - tile-granular engine assignment beats
    any finer column split on hardware (fewer instructions, less sync).
  * L3: per group of 32 rows, 64 f16 matmuls with zero-padded [128, 32]
    w3/16 stationary views accumulate score rows into one [32, 512] PSUM
    tile (engine partition writes must be 32-aligned); one ACT copy per
    group scatters it into the [64, 512] scores tile.
  * Endgame: max-subtracted logsumexp over x (ACT exp with accum_out) +
    masked diagonal extraction -> [64, 2] per core.

Pool (gpsimd) is deliberately unused in the main loop: its tensor ops
measure ~2 us each on this device (software DSP path), far off the cost
model, and they degrade further inside hardware For_i loops.
"""

import sys

import numpy as np

_TRN_REPO = "/opt/trn_rl_repo"
if _TRN_REPO not in sys.path:
    sys.path.insert(0, _TRN_REPO)

B = 512
NX = 64
NY = 64
HID = 256
N_CORES = 8
SH = B // N_CORES  # y rows per core
GROUP = 32         # rows per L3 psum group

_PROG_CACHE = {}


def _emit(
    tc,
    aps,
    repeat=None,
    zp=0,         # h1 chunk0 cols on Pool (rest DVE; Pool is slow on hw)
    zp2=0,        # h1 chunk1 cols on Pool (rest DVE)
    sa0=512,      # h2 m=0 cols on ACT (rest DVE)
    sa1=0,        # h2 m=1 cols on ACT (rest DVE)
    h1a=0,        # h1 chunk0 cols on ACT (rest per zp/DVE)
):
    import contextlib

    import concourse.bass as bass  # noqa: F401
    from concourse import mybir

    nc = tc.nc
    f32 = mybir.dt.float32
    f16 = mybir.dt.float16
    f8 = mybir.dt.float8e4
    AF = mybir.ActivationFunctionType
    ALU = mybir.AluOpType
    AX = mybir.AxisListType

    with (
        tc.tile_pool(name="const", bufs=1) as cpool,
        tc.tile_pool(name="work", bufs=3) as wpool,
        tc.tile_pool(name="psum", bufs=2, space="PSUM") as ppool,
    ):
        # ---------------- persistent loads ----------------
        xt = cpool.tile([NX, B], f32, name="xt_sb")
        nc.sync.dma_start(xt[:], aps["xt"][:])
        yt = cpool.tile([NY, SH], f32, name="yt_sb")
        nc.sync.dma_start(yt[:], aps["yt"][:])
        w1x = cpool.tile([NX, HID], f32, name="w1x_sb")
        nc.sync.dma_start(w1x[:], aps["w1"][0:NX, :])
        w1y = cpool.tile([NY, HID], f32, name="w1y_sb")
        nc.sync.dma_start(w1y[:], aps["w1"][NX : NX + NY, :])
        b1c = cpool.tile([128, 2], f32, name="b1_sb")
        nc.sync.dma_start(b1c[:], aps["b1"].rearrange("(k p) -> p k", p=128))
        b2c = cpool.tile([128, 2], f32, name="b2_sb")
        nc.sync.dma_start(b2c[:], aps["b2"].rearrange("(k p) -> p k", p=128))
        b2x16 = cpool.tile([128, 2], f32, name="b2x16_sb")
        nc.sync.dma_start(b2x16[:], aps["b2x16"].rearrange("(k p) -> p k", p=128))
        w2hi = cpool.tile([128, 512], f8, name="w2hi_sb")
        nc.sync.dma_start(w2hi[:], aps["w2hi"][:])
        w2lo = cpool.tile([128, 512], f8, name="w2lo_sb")
        nc.sync.dma_start(w2lo[:], aps["w2lo"][:])
        w3pad = cpool.tile([128, 126], f16, name="w3pad_sb")
        nc.sync.dma_start(w3pad[:], aps["w3pad"][:])
        mask = cpool.tile([SH, B], f32, name="mask_sb")
        nc.sync.dma_start(mask[:], aps["mask"][:])

        scores = cpool.tile([SH, B], f32, name="scores_sb")

        # ---------------- layer-1 precompute ----------------
        a16 = []
        cb = []
        for m in range(2):
            pa = ppool.tile([128, B], f32, tag="p2_0", name=f"pa_{m}", bufs=2)
            nc.tensor.matmul(
                pa[:], w1x[:, 128 * m : 128 * m + 128], xt[:], start=True, stop=True
            )
            a = cpool.tile([128, B], f16, name=f"a16_{m}")
            nc.scalar.copy(a[:], pa[:])
            a16.append(a)

            pc = ppool.tile([128, SH], f32, tag="p2_1", name=f"pc_{m}", bufs=2)
            nc.tensor.matmul(
                pc[:], w1y[:, 128 * m : 128 * m + 128], yt[:], start=True, stop=True
            )
            c = cpool.tile([128, SH], f32, name=f"cb_{m}")
            nc.scalar.activation(c[:], pc[:], AF.Identity, bias=b1c[:, m : m + 1])
            cb.append(c)

        # DR operand views of the weight tiles: [128, 2, 128] per hid-out block
        hi_v = [
            w2hi[:, 256 * mb : 256 * mb + 256].rearrange("p (k m) -> p k m", k=2)
            for mb in range(2)
        ]
        lo_v = [
            w2lo[:, 256 * mb : 256 * mb + 256].rearrange("p (k m) -> p k m", k=2)
            for mb in range(2)
        ]

        # ---------------- main loop (software-pipelined) ----------------
        # slot t: h1(t+1) | DR-L2(t) | h2-relu(t-1) | L3(t-2)
        # so every instruction consumes only prior-slot results: no engine
        # ever stalls on a same-slot cross-engine dependency.
        loop_cm = (
            tc.For_i(0, repeat, 1)
            if repeat is not None and repeat > 1
            else contextlib.nullcontext()
        )
        n = SH
        with loop_cm:
            h1s, p2s, h2s, pscs = {}, {}, {}, {}

            def emit_h1(i):
                h1 = wpool.tile([128, 1024], f8, tag="h1", name=f"h1_{i}", bufs=3)
                for c, z in ((0, zp), (1, zp2)):
                    base = 512 * c
                    a = h1a if c == 0 else 0
                    if a > 0:
                        nc.scalar.activation(
                            h1[:, base : base + a], a16[c][:, 0:a], AF.Relu,
                            bias=cb[c][:, i : i + 1],
                        )
                    if z > 0:
                        nc.gpsimd.tensor_scalar(
                            h1[:, base + a : base + a + z], a16[c][:, a : a + z],
                            cb[c][:, i : i + 1], 0.0, ALU.add, ALU.max,
                        )
                    if a + z < 512:
                        nc.vector.tensor_scalar(
                            h1[:, base + a + z : base + 512], a16[c][:, a + z : 512],
                            cb[c][:, i : i + 1], 0.0, ALU.add, ALU.max,
                        )
                h1s[i] = h1

            def emit_dr(i):
                h1v = h1s.pop(i)[:].rearrange("p (k n) -> p k n", k=2)
                ps = []
                for m in range(2):
                    p2 = ppool.tile(
                        [128, B], f32, tag=f"p2_{m}", name=f"p2_{m}_{i}", bufs=2
                    )
                    nc.tensor.matmul(
                        p2[:], hi_v[m], h1v, start=True, stop=False,
                        perf_mode=mybir.MatmulPerfMode.DoubleRow,
                    )
                    nc.tensor.matmul(
                        p2[:], lo_v[m], h1v, start=False, stop=True,
                        perf_mode=mybir.MatmulPerfMode.DoubleRow,
                    )
                    ps.append(p2)
                p2s[i] = ps

            def emit_h2(i):
                ps = p2s.pop(i)
                hs = []
                # all h2 slices hold 16*h2 (bias 16*b2, no descale); the
                # global 1/16 is folded into w3pad host-side.
                for m, sa in ((0, sa0), (1, sa1)):
                    p2 = ps[m]
                    h2 = wpool.tile(
                        [128, B], f16, tag=f"h2_{m}", name=f"h2_{m}_{i}", bufs=3
                    )
                    if sa > 0:
                        nc.scalar.activation(
                            h2[:, 0:sa], p2[:, 0:sa], AF.Relu,
                            bias=b2x16[:, m : m + 1],
                        )
                    if sa < 512:
                        nc.vector.tensor_scalar(
                            h2[:, sa:512], p2[:, sa:512],
                            b2x16[:, m : m + 1], 0.0, ALU.add, ALU.max,
                        )
                    hs.append(h2)
                h2s[i] = hs

            def emit_l3(i):
                g, j = divmod(i, GROUP)
                if j == 0:
                    pscs[g] = ppool.tile(
                        [GROUP, B], f32, tag="psc", name=f"psc_{g}", bufs=2
                    )
                psc = pscs[g]
                hs = h2s.pop(i)
                for m in range(2):
                    nc.tensor.matmul(
                        psc[:],
                        w3pad[:, 63 * m + 31 - j : 63 * m + 63 - j],
                        hs[m][:],
                        start=(j == 0 and m == 0),
                        stop=(j == GROUP - 1 and m == 1),
                        skip_group_check=True,
                    )
                if j == GROUP - 1:
                    psc = pscs.pop(g)
                    r0 = GROUP * g
                    nc.scalar.copy(scores[r0 : r0 + GROUP, :], psc[:])

            emit_h1(0)
            for t in range(n + 2):
                if t + 1 < n:
                    emit_h1(t + 1)
                if t < n:
                    emit_dr(t)
                if 1 <= t <= n:
                    emit_h2(t - 1)
                if t >= 2:
                    emit_l3(t - 2)

        # ---------------- logsumexp + diag ----------------
        otile = cpool.tile([SH, 2], f32, name="otile")
        negmax = cpool.tile([SH, 1], f32, name="negmax")
        nc.vector.tensor_reduce(negmax[:], scores[:], AX.X, ALU.max, negate=True)
        expt = cpool.tile([SH, B], f32, name="expt")
        sumexp = cpool.tile([SH, 1], f32, name="sumexp")
        nc.scalar.activation(
            expt[:], scores[:], AF.Exp, bias=negmax[:], accum_out=sumexp[:]
        )
        lse0 = cpool.tile([SH, 1], f32, name="lse0")
        nc.scalar.activation(lse0[:], sumexp[:], AF.Ln)
        nc.vector.tensor_scalar(
            otile[:, 0:1], lse0[:], negmax[:], None, ALU.subtract
        )
        mjunk = cpool.tile([SH, B], f32, name="mjunk")
        nc.vector.tensor_mul(mjunk[:], scores[:], mask[:])
        nc.vector.tensor_reduce(otile[:, 1:2], mjunk[:], AX.X, ALU.add)
        nc.sync.dma_start(aps["out"][:], otile[:])


def _declare(nc):
    from concourse import mybir

    f32 = mybir.dt.float32
    f16 = mybir.dt.float16
    f8 = mybir.dt.float8e4
    return {
        "xt": nc.dram_tensor("xt", [NX, B], f32, kind="ExternalInput").ap(),
        "yt": nc.dram_tensor("yt", [NY, SH], f32, kind="ExternalInput").ap(),
        "w1": nc.dram_tensor("w1", [NX + NY, HID], f32, kind="ExternalInput").ap(),
        "b1": nc.dram_tensor("b1", [HID], f32, kind="ExternalInput").ap(),
        "b2": nc.dram_tensor("b2", [HID], f32, kind="ExternalInput").ap(),
        "b2x16": nc.dram_tensor("b2x16", [HID], f32, kind="ExternalInput").ap(),
        "w2hi": nc.dram_tensor("w2hi", [128, 512], f8, kind="ExternalInput").ap(),
        "w2lo": nc.dram_tensor("w2lo", [128, 512], f8, kind="ExternalInput").ap(),
        "w3pad": nc.dram_tensor("w3pad", [128, 126], f16, kind="ExternalInput").ap(),
        "mask": nc.dram_tensor("mask", [SH, B], f32, kind="ExternalInput").ap(),
        "out": nc.dram_tensor("out", [SH, 2], f32, kind="ExternalOutput").ap(),
    }


def build_program(repeat=None, **emit_kwargs):
    import concourse.tile as tile
    from concourse import bacc

    nc = bacc.Bacc(
        "TRN2", target_bir_lowering=False, debug=False, num_devices=N_CORES
    )
    aps = _declare(nc)
    with tile.TileContext(nc) as tc:
        _emit(tc, aps, repeat=repeat, **emit_kwargs)
    nc.compile()
    return nc


def _get_program():
    if "nc" in _PROG_CACHE:
        return _PROG_CACHE["nc"]
    nc = build_program()
    _PROG_CACHE["nc"] = nc
    return nc


def _make_in_maps(dataX, dataY, W1, b1, W2, b2, W3):
    import ml_dtypes

    F8 = ml_dtypes.float8_e4m3fn

    dataX = np.asarray(dataX, np.float32)
    dataY = np.asarray(dataY, np.float32)
    W1 = np.asarray(W1, np.float32)
    b1 = np.asarray(b1, np.float32)
    W2 = np.asarray(W2, np.float32)
    b2 = np.asarray(b2, np.float32)
    W3 = np.asarray(W3, np.float32)

    xt = np.ascontiguousarray(dataX.T)

    # W2 hi+lo fp8 at common x16 scale, k-major DR layout
    T16 = (16.0 * W2).astype(F8)
    Rlo = (16.0 * W2 - T16.astype(np.float32)).astype(F8)
    w2hi = np.zeros((128, 512), F8)
    w2lo = np.zeros((128, 512), F8)
    for mb in range(2):
        for k in range(2):
            col = 256 * mb + 128 * k
            w2hi[:, col : col + 128] = T16[128 * k : 128 * (k + 1),
                                           128 * mb : 128 * (mb + 1)]
            w2lo[:, col : col + 128] = Rlo[128 * k : 128 * (k + 1),
                                           128 * mb : 128 * (mb + 1)]

    # zero-padded w3 stationary strips: [128, 31] per chunk, col 15 = w3 chunk
    w3pad = np.zeros((128, 126), np.float16)
    w3pad[:, 31] = (W3[0:128, 0] / 16.0).astype(np.float16)
    w3pad[:, 63 + 31] = (W3[128:256, 0] / 16.0).astype(np.float16)

    in_maps = []
    for c in range(N_CORES):
        ytc = np.ascontiguousarray(dataY[c * SH : (c + 1) * SH].T)
        maskc = np.zeros((SH, B), np.float32)
        maskc[np.arange(SH), c * SH + np.arange(SH)] = 1.0
        in_maps.append(
            {
                "xt": xt,
                "yt": ytc,
                "w1": W1,
                "b1": b1,
                "b2": b2,
                "b2x16": 16.0 * b2,
                "w2hi": w2hi,
                "w2lo": w2lo,
                "w3pad": w3pad,
                "mask": maskc,
            }
        )
    return in_maps


def _combine(results):
    lse = np.concatenate([np.asarray(r["out"])[:, 0] for r in results])
    diag = np.concatenate([np.asarray(r["out"])[:, 1] for r in results])
    log_b = np.log(np.float64(B))
    mi = log_b + diag.astype(np.float64).mean() - lse.astype(np.float64).mean()
    return np.asarray(-mi, dtype=np.float32)


def _run(inputs):
    import time

    from concourse import bass_utils

    nc = _get_program()
    in_maps = _make_in_maps(
        inputs["dataX"],
        inputs["dataY"],
        inputs["W1"],
        inputs["b1"],
        inputs["W2"],
        inputs["b2"],
        inputs["W3"],
    )
    last_exc = None
    for attempt in range(4):
        try:
            res = bass_utils.run_bass_kernel_spmd(
                nc, in_maps, core_ids=list(range(N_CORES)), trace=False
            )
            out = _combine(res.results)
            if np.isfinite(out):
                return out, res
            last_exc = RuntimeError("non-finite kernel output")
        except Exception as exc:  # noqa: BLE001
            last_exc = exc
        time.sleep(2.0 * (attempt + 1))
        try:
            import jax

            jax.clear_caches()
        except Exception:  # noqa: BLE001
            pass
    raise last_exc


class _Executor:
    """Reusable sharded executable over the 8 cores, for timing loops.

    Replicates bass2jax.run_bass_via_pjrt's multi-core path but keeps the
    jitted callable and device-resident inputs so repeated calls measure
    dispatch + NEFF execution only (no fresh trace/compile, no host->device
    input transfer).
    """

    def __init__(self, nc, in_maps):
        import jax
        import numpy as np
        from jax.sharding import Mesh, NamedSharding, PartitionSpec
        from jax.experimental.shard_map import shard_map

        from concourse import bass2jax, mybir

        bass2jax.install_neuronx_cc_hook()

        partition_name = (
            nc.partition_id_tensor.name if nc.partition_id_tensor else None
        )
        in_names, out_names, out_avals, zero_outs = [], [], [], []
        for alloc in nc.m.functions[0].allocations:
            if not isinstance(alloc, mybir.MemoryLocationSet):
                continue
            name = alloc.memorylocations[0].name
            if alloc.kind == "ExternalInput":
                if name != partition_name:
                    in_names.append(name)
            elif alloc.kind == "ExternalOutput":
                out_names.append(name)
                shape = tuple(alloc.tensor_shape)
                dtype = mybir.dt.np(alloc.dtype)
                out_avals.append(jax.core.ShapedArray(shape, dtype))
                zero_outs.append(np.zeros(shape, dtype))
        n_params = len(in_names)
        n_outs = len(out_avals)
        all_in_names = list(in_names) + list(out_names)
        if partition_name is not None:
            all_in_names.append(partition_name)
        donate = tuple(range(n_params, n_params + n_outs))

        def _body(*args):
            operands = list(args)
            if partition_name is not None:
                operands.append(bass2jax.partition_id_tensor())
            outs = bass2jax._bass_exec_p.bind(
                *operands,
                out_avals=tuple(out_avals),
                in_names=tuple(all_in_names),
                out_names=tuple(out_names),
                lowering_input_output_aliases=(),
                sim_require_finite=True,
                sim_require_nnan=True,
                nc=nc,
            )
            return tuple(outs)

        devices = jax.devices()[:N_CORES]
        mesh = Mesh(np.asarray(devices), ("core",))
        in_specs = (PartitionSpec("core"),) * (n_params + n_outs)
        out_specs = (PartitionSpec("core"),) * len(out_names)
        self._fn = jax.jit(
            shard_map(
                _body,
                mesh=mesh,
                in_specs=in_specs,
                out_specs=out_specs,
                check_rep=False,
            ),
            donate_argnums=donate,
            keep_unused=True,
        )
        per_core = [
            [np.asarray(m[name]) for name in in_names] for m in in_maps
        ]
        sharding = NamedSharding(mesh, PartitionSpec("core"))
        self._dev_in = [
            jax.device_put(
                np.concatenate([per_core[c][i] for c in range(N_CORES)], axis=0),
                sharding,
            )
            for i in range(n_params)
        ]
        self._zero_shapes = [
            ((N_CORES * z.shape[0],) + z.shape[1:], z.dtype) for z in zero_outs
        ]
        self._out_names = out_names
        self._out_avals = out_avals
        self._jax = jax

    def __call__(self):
        zeros = [np.zeros(s, d) for s, d in self._zero_shapes]
        outs = self._fn(*self._dev_in, *zeros)
        self._jax.block_until_ready(outs)
        return outs

    def results(self, outs):
        res = []
        for c in range(N_CORES):
            res.append(
                {
                    name: np.asarray(outs[i]).reshape(
                        N_CORES, *self._out_avals[i].shape
                    )[c]
                    for i, name in enumerate(self._out_names)
                }
            )
        return res


def kernel(**inputs):
    return _run(inputs)[0]


# revision 6
# speedup vs baseline: 2.8947x; 2.8947x over previous
"""Trainium2 Bass kernel for the DSIB InfoNCE loss (fp8 DoubleRow).

Data-parallel over the 512 y rows: 8 cores x 64 rows; each core computes
its [64, 512] score block, reduces it to per-row (logsumexp, diag) pairs,
and the host sums the partials (the all-reduce of the sharding hint).

Per-core pipeline (software-pipelined slots: h1(t+1) | L2(t) | relu(t-1)
| L3(t-2), so no engine waits on a same-slot cross-engine dependency):
  * L1 linearity: A = X @ W1x is precomputed once ([256, 512] f16,
    hid-in on partitions); c_i = Y_i @ W1y + b1 per row.  DVE writes
    h1 = relu(A + c_i) straight into a [128, 1024] fp8(e4m3) tile (the
    two 128-channel hid-in chunks side by side = the k-major DoubleRow
    moving layout).
  * L2 runs as fp8 DoubleRow matmuls (K=256 contracted in one pass via
    [128, 2, *] operand views).  W2 ships as an fp8 hi+lo pair at a
    common x16 scale accumulating in one PSUM tile: W2's quantization
    error dominates the fp8 loss error, while h1's own fp8 error largely
    cancels between the diag and lse means, so compensating W2 alone
    holds the final rel err at ~4e-3 (vs ~2e-2 uncompensated).
  * h2 = relu(z16 + 16*b2) -> f16 (values x16; the global 1/16 is folded
    into the L3 weights).  One ACT instruction for hid-out block 0, one
    DVE instruction for block 1 - tile-granular engine assignment beats
    any finer column split on hardware (fewer instructions, less sync).
  * L3: per group of 32 rows, 64 f16 matmuls with zero-padded [128, 32]
    w3/16 stationary views accumulate score rows into one [32, 512] PSUM
    tile (engine partition writes must be 32-aligned); one ACT copy per
    group scatters it into the [64, 512] scores tile.
  * Endgame: max-subtracted logsumexp over x (ACT exp with accum_out) +
    masked diagonal extraction -> [64, 2] per core.

Pool (gpsimd) is deliberately unused in the main loop: its tensor ops
measure ~2 us each on this device (software DSP path), far off the cost
model, and they degrade further inside hardware For_i loops.
"""

import sys

import numpy as np

_TRN_REPO = "/opt/trn_rl_repo"
if _TRN_REPO not in sys.path:
    sys.path.insert(0, _TRN_REPO)

B = 512
NX = 64
NY = 64
HID = 256
N_CORES = 8
SH = B // N_CORES  # y rows per core
GROUP = 32         # rows per L3 psum group

_PROG_CACHE = {}


def _emit(
    tc,
    aps,
    repeat=None,
    zp=0,         # h1 chunk0 cols on Pool (rest DVE; Pool is slow on hw)
    zp2=0,        # h1 chunk1 cols on Pool (rest DVE)
    sa0=512,      # h2 m=0 cols on ACT (rest DVE)
    sa1=0,        # h2 m=1 cols on ACT (rest DVE)
    h1a=512,      # h1 chunk0 cols on ACT (rest per zp/DVE)
    p2b=2,        # p2 psum bufs per tag (2 tags; total psum banks = 2*p2b + 2)
    wb=3,         # h1/h2 sbuf bufs
):
    import contextlib

    import concourse.bass as bass  # noqa: F401
    from concourse import mybir

    nc = tc.nc
    f32 = mybir.dt.float32
    f16 = mybir.dt.float16
    f8 = mybir.dt.float8e4
    AF = mybir.ActivationFunctionType
    ALU = mybir.AluOpType
    AX = mybir.AxisListType

    with (
        tc.tile_pool(name="const", bufs=1) as cpool,
        tc.tile_pool(name="work", bufs=3) as wpool,
        tc.tile_pool(name="psum", bufs=2, space="PSUM") as ppool,
    ):
        # ---------------- persistent loads ----------------
        xt = cpool.tile([NX, B], f32, name="xt_sb")
        nc.sync.dma_start(xt[:], aps["xt"][:])
        yt = cpool.tile([NY, SH], f32, name="yt_sb")
        nc.sync.dma_start(yt[:], aps["yt"][:])
        w1x = cpool.tile([NX, HID], f32, name="w1x_sb")
        nc.sync.dma_start(w1x[:], aps["w1"][0:NX, :])
        w1y = cpool.tile([NY, HID], f32, name="w1y_sb")
        nc.sync.dma_start(w1y[:], aps["w1"][NX : NX + NY, :])
        b1c = cpool.tile([128, 2], f32, name="b1_sb")
        nc.sync.dma_start(b1c[:], aps["b1"].rearrange("(k p) -> p k", p=128))
        b2c = cpool.tile([128, 2], f32, name="b2_sb")
        nc.sync.dma_start(b2c[:], aps["b2"].rearrange("(k p) -> p k", p=128))
        b2x16 = cpool.tile([128, 2], f32, name="b2x16_sb")
        nc.sync.dma_start(b2x16[:], aps["b2x16"].rearrange("(k p) -> p k", p=128))
        w2hi = cpool.tile([128, 512], f8, name="w2hi_sb")
        nc.sync.dma_start(w2hi[:], aps["w2hi"][:])
        w2lo = cpool.tile([128, 512], f8, name="w2lo_sb")
        nc.sync.dma_start(w2lo[:], aps["w2lo"][:])
        w3pad = cpool.tile([128, 126], f16, name="w3pad_sb")
        nc.sync.dma_start(w3pad[:], aps["w3pad"][:])
        mask = cpool.tile([SH, B], f32, name="mask_sb")
        nc.sync.dma_start(mask[:], aps["mask"][:])

        scores = cpool.tile([SH, B], f32, name="scores_sb")

        # ---------------- layer-1 precompute ----------------
        a16 = []
        cb = []
        for m in range(2):
            pa = ppool.tile([128, B], f32, tag="p2_0", name=f"pa_{m}", bufs=p2b)
            nc.tensor.matmul(
                pa[:], w1x[:, 128 * m : 128 * m + 128], xt[:], start=True, stop=True
            )
            a = cpool.tile([128, B], f16, name=f"a16_{m}")
            nc.scalar.copy(a[:], pa[:])
            a16.append(a)

            pc = ppool.tile([128, SH], f32, tag="p2_1", name=f"pc_{m}", bufs=p2b)
            nc.tensor.matmul(
                pc[:], w1y[:, 128 * m : 128 * m + 128], yt[:], start=True, stop=True
            )
            c = cpool.tile([128, SH], f32, name=f"cb_{m}")
            nc.scalar.activation(c[:], pc[:], AF.Identity, bias=b1c[:, m : m + 1])
            cb.append(c)

        # DR operand views of the weight tiles: [128, 2, 128] per hid-out block
        hi_v = [
            w2hi[:, 256 * mb : 256 * mb + 256].rearrange("p (k m) -> p k m", k=2)
            for mb in range(2)
        ]
        lo_v = [
            w2lo[:, 256 * mb : 256 * mb + 256].rearrange("p (k m) -> p k m", k=2)
            for mb in range(2)
        ]

        # ---------------- main loop (software-pipelined) ----------------
        # slot t: h1(t+1) | DR-L2(t) | h2-relu(t-1) | L3(t-2)
        # so every instruction consumes only prior-slot results: no engine
        # ever stalls on a same-slot cross-engine dependency.
        loop_cm = (
            tc.For_i(0, repeat, 1)
            if repeat is not None and repeat > 1
            else contextlib.nullcontext()
        )
        n = SH
        with loop_cm:
            h1s, p2s, h2s, pscs = {}, {}, {}, {}

            def emit_h1(i):
                h1 = wpool.tile([128, 1024], f8, tag="h1", name=f"h1_{i}", bufs=wb)
                for c, z in ((0, zp), (1, zp2)):
                    base = 512 * c
                    a = h1a if c == 0 else 0
                    if a > 0:
                        nc.scalar.activation(
                            h1[:, base : base + a], a16[c][:, 0:a], AF.Relu,
                            bias=cb[c][:, i : i + 1],
                        )
                    if z > 0:
                        nc.gpsimd.tensor_scalar(
                            h1[:, base + a : base + a + z], a16[c][:, a : a + z],
                            cb[c][:, i : i + 1], 0.0, ALU.add, ALU.max,
                        )
                    if a + z < 512:
                        nc.vector.tensor_scalar(
                            h1[:, base + a + z : base + 512], a16[c][:, a + z : 512],
                            cb[c][:, i : i + 1], 0.0, ALU.add, ALU.max,
                        )
                h1s[i] = h1

            def emit_dr(i):
                h1v = h1s.pop(i)[:].rearrange("p (k n) -> p k n", k=2)
                ps = []
                for m in range(2):
                    p2 = ppool.tile(
                        [128, B], f32, tag=f"p2_{m}", name=f"p2_{m}_{i}", bufs=p2b
                    )
                    nc.tensor.matmul(
                        p2[:], hi_v[m], h1v, start=True, stop=False,
                        perf_mode=mybir.MatmulPerfMode.DoubleRow,
                    )
                    nc.tensor.matmul(
                        p2[:], lo_v[m], h1v, start=False, stop=True,
                        perf_mode=mybir.MatmulPerfMode.DoubleRow,
                    )
                    ps.append(p2)
                p2s[i] = ps

            def emit_h2(i):
                ps = p2s.pop(i)
                hs = []
                # all h2 slices hold 16*h2 (bias 16*b2, no descale); the
                # global 1/16 is folded into w3pad host-side.
                for m, sa in ((0, sa0), (1, sa1)):
                    p2 = ps[m]
                    h2 = wpool.tile(
                        [128, B], f16, tag=f"h2_{m}", name=f"h2_{m}_{i}", bufs=wb
                    )
                    if sa > 0:
                        nc.scalar.activation(
                            h2[:, 0:sa], p2[:, 0:sa], AF.Relu,
                            bias=b2x16[:, m : m + 1],
                        )
                    if sa < 512:
                        nc.vector.tensor_scalar(
                            h2[:, sa:512], p2[:, sa:512],
                            b2x16[:, m : m + 1], 0.0, ALU.add, ALU.max,
                        )
                    hs.append(h2)
                h2s[i] = hs

            def emit_l3(i):
                g, j = divmod(i, GROUP)
                if j == 0:
                    pscs[g] = ppool.tile(
                        [GROUP, B], f32, tag="psc", name=f"psc_{g}", bufs=2
                    )
                psc = pscs[g]
                hs = h2s.pop(i)
                for m in range(2):
                    nc.tensor.matmul(
                        psc[:],
                        w3pad[:, 63 * m + 31 - j : 63 * m + 63 - j],
                        hs[m][:],
                        start=(j == 0 and m == 0),
                        stop=(j == GROUP - 1 and m == 1),
                        skip_group_check=True,
                    )
                if j == GROUP - 1:
                    psc = pscs.pop(g)
                    r0 = GROUP * g
                    nc.scalar.copy(scores[r0 : r0 + GROUP, :], psc[:])

            emit_h1(0)
            for t in range(n + 2):
                if t + 1 < n:
                    emit_h1(t + 1)
                if t < n:
                    emit_dr(t)
                if 1 <= t <= n:
                    emit_h2(t - 1)
                if t >= 2:
                    emit_l3(t - 2)

        # ---------------- logsumexp + diag ----------------
        otile = cpool.tile([SH, 2], f32, name="otile")
        negmax = cpool.tile([SH, 1], f32, name="negmax")
        nc.vector.tensor_reduce(negmax[:], scores[:], AX.X, ALU.max, negate=True)
        expt = cpool.tile([SH, B], f32, name="expt")
        sumexp = cpool.tile([SH, 1], f32, name="sumexp")
        nc.scalar.activation(
            expt[:], scores[:], AF.Exp, bias=negmax[:], accum_out=sumexp[:]
        )
        lse0 = cpool.tile([SH, 1], f32, name="lse0")
        nc.scalar.activation(lse0[:], sumexp[:], AF.Ln)
        nc.vector.tensor_scalar(
            otile[:, 0:1], lse0[:], negmax[:], None, ALU.subtract
        )
        mjunk = cpool.tile([SH, B], f32, name="mjunk")
        nc.vector.tensor_mul(mjunk[:], scores[:], mask[:])
        nc.vector.tensor_reduce(otile[:, 1:2], mjunk[:], AX.X, ALU.add)
        nc.sync.dma_start(aps["out"][:], otile[:])


def _declare(nc):
    from concourse import mybir

    f32 = mybir.dt.float32
    f16 = mybir.dt.float16
    f8 = mybir.dt.float8e4
    return {
        "xt": nc.dram_tensor("xt", [NX, B], f32, kind="ExternalInput").ap(),
        "yt": nc.dram_tensor("yt", [NY, SH], f32, kind="ExternalInput").ap(),
        "w1": nc.dram_tensor("w1", [NX + NY, HID], f32, kind="ExternalInput").ap(),
        "b1": nc.dram_tensor("b1", [HID], f32, kind="ExternalInput").ap(),
        "b2": nc.dram_tensor("b2", [HID], f32, kind="ExternalInput").ap(),
        "b2x16": nc.dram_tensor("b2x16", [HID], f32, kind="ExternalInput").ap(),
        "w2hi": nc.dram_tensor("w2hi", [128, 512], f8, kind="ExternalInput").ap(),
        "w2lo": nc.dram_tensor("w2lo", [128, 512], f8, kind="ExternalInput").ap(),
        "w3pad": nc.dram_tensor("w3pad", [128, 126], f16, kind="ExternalInput").ap(),
        "mask": nc.dram_tensor("mask", [SH, B], f32, kind="ExternalInput").ap(),
        "out": nc.dram_tensor("out", [SH, 2], f32, kind="ExternalOutput").ap(),
    }


def build_program(repeat=None, **emit_kwargs):
    import concourse.tile as tile
    from concourse import bacc

    nc = bacc.Bacc(
        "TRN2", target_bir_lowering=False, debug=False, num_devices=N_CORES
    )
    aps = _declare(nc)
    with tile.TileContext(nc) as tc:
        _emit(tc, aps, repeat=repeat, **emit_kwargs)
    nc.compile()
    return nc


def _get_program():
    if "nc" in _PROG_CACHE:
        return _PROG_CACHE["nc"]
    nc = build_program()
    _PROG_CACHE["nc"] = nc
    return nc


def _make_in_maps(dataX, dataY, W1, b1, W2, b2, W3):
    import ml_dtypes

    F8 = ml_dtypes.float8_e4m3fn

    dataX = np.asarray(dataX, np.float32)
    dataY = np.asarray(dataY, np.float32)
    W1 = np.asarray(W1, np.float32)
    b1 = np.asarray(b1, np.float32)
    W2 = np.asarray(W2, np.float32)
    b2 = np.asarray(b2, np.float32)
    W3 = np.asarray(W3, np.float32)

    xt = np.ascontiguousarray(dataX.T)

    # W2 hi+lo fp8 at common x16 scale, k-major DR layout
    T16 = (16.0 * W2).astype(F8)
    Rlo = (16.0 * W2 - T16.astype(np.float32)).astype(F8)
    w2hi = np.zeros((128, 512), F8)
    w2lo = np.zeros((128, 512), F8)
    for mb in range(2):
        for k in range(2):
            col = 256 * mb + 128 * k
            w2hi[:, col : col + 128] = T16[128 * k : 128 * (k + 1),
                                           128 * mb : 128 * (mb + 1)]
            w2lo[:, col : col + 128] = Rlo[128 * k : 128 * (k + 1),
                                           128 * mb : 128 * (mb + 1)]

    # zero-padded w3 stationary strips: [128, 31] per chunk, col 15 = w3 chunk
    w3pad = np.zeros((128, 126), np.float16)
    w3pad[:, 31] = (W3[0:128, 0] / 16.0).astype(np.float16)
    w3pad[:, 63 + 31] = (W3[128:256, 0] / 16.0).astype(np.float16)

    in_maps = []
    for c in range(N_CORES):
        ytc = np.ascontiguousarray(dataY[c * SH : (c + 1) * SH].T)
        maskc = np.zeros((SH, B), np.float32)
        maskc[np.arange(SH), c * SH + np.arange(SH)] = 1.0
        in_maps.append(
            {
                "xt": xt,
                "yt": ytc,
                "w1": W1,
                "b1": b1,
                "b2": b2,
                "b2x16": 16.0 * b2,
                "w2hi": w2hi,
                "w2lo": w2lo,
                "w3pad": w3pad,
                "mask": maskc,
            }
        )
    return in_maps


def _combine(results):
    lse = np.concatenate([np.asarray(r["out"])[:, 0] for r in results])
    diag = np.concatenate([np.asarray(r["out"])[:, 1] for r in results])
    log_b = np.log(np.float64(B))
    mi = log_b + diag.astype(np.float64).mean() - lse.astype(np.float64).mean()
    return np.asarray(-mi, dtype=np.float32)


def _run(inputs):
    import time

    from concourse import bass_utils

    nc = _get_program()
    in_maps = _make_in_maps(
        inputs["dataX"],
        inputs["dataY"],
        inputs["W1"],
        inputs["b1"],
        inputs["W2"],
        inputs["b2"],
        inputs["W3"],
    )
    last_exc = None
    for attempt in range(4):
        try:
            res = bass_utils.run_bass_kernel_spmd(
                nc, in_maps, core_ids=list(range(N_CORES)), trace=False
            )
            out = _combine(res.results)
            if np.isfinite(out):
                return out, res
            last_exc = RuntimeError("non-finite kernel output")
        except Exception as exc:  # noqa: BLE001
            last_exc = exc
        time.sleep(2.0 * (attempt + 1))
        try:
            import jax

            jax.clear_caches()
        except Exception:  # noqa: BLE001
            pass
    raise last_exc


class _Executor:
    """Reusable sharded executable over the 8 cores, for timing loops.

    Replicates bass2jax.run_bass_via_pjrt's multi-core path but keeps the
    jitted callable and device-resident inputs so repeated calls measure
    dispatch + NEFF execution only (no fresh trace/compile, no host->device
    input transfer).
    """

    def __init__(self, nc, in_maps):
        import jax
        import numpy as np
        from jax.sharding import Mesh, NamedSharding, PartitionSpec
        from jax.experimental.shard_map import shard_map

        from concourse import bass2jax, mybir

        bass2jax.install_neuronx_cc_hook()

        partition_name = (
            nc.partition_id_tensor.name if nc.partition_id_tensor else None
        )
        in_names, out_names, out_avals, zero_outs = [], [], [], []
        for alloc in nc.m.functions[0].allocations:
            if not isinstance(alloc, mybir.MemoryLocationSet):
                continue
            name = alloc.memorylocations[0].name
            if alloc.kind == "ExternalInput":
                if name != partition_name:
                    in_names.append(name)
            elif alloc.kind == "ExternalOutput":
                out_names.append(name)
                shape = tuple(alloc.tensor_shape)
                dtype = mybir.dt.np(alloc.dtype)
                out_avals.append(jax.core.ShapedArray(shape, dtype))
                zero_outs.append(np.zeros(shape, dtype))
        n_params = len(in_names)
        n_outs = len(out_avals)
        all_in_names = list(in_names) + list(out_names)
        if partition_name is not None:
            all_in_names.append(partition_name)
        donate = tuple(range(n_params, n_params + n_outs))

        def _body(*args):
            operands = list(args)
            if partition_name is not None:
                operands.append(bass2jax.partition_id_tensor())
            outs = bass2jax._bass_exec_p.bind(
                *operands,
                out_avals=tuple(out_avals),
                in_names=tuple(all_in_names),
                out_names=tuple(out_names),
                lowering_input_output_aliases=(),
                sim_require_finite=True,
                sim_require_nnan=True,
                nc=nc,
            )
            return tuple(outs)

        devices = jax.devices()[:N_CORES]
        mesh = Mesh(np.asarray(devices), ("core",))
        in_specs = (PartitionSpec("core"),) * (n_params + n_outs)
        out_specs = (PartitionSpec("core"),) * len(out_names)
        self._fn = jax.jit(
            shard_map(
                _body,
                mesh=mesh,
                in_specs=in_specs,
                out_specs=out_specs,
                check_rep=False,
            ),
            donate_argnums=donate,
            keep_unused=True,
        )
        per_core = [
            [np.asarray(m[name]) for name in in_names] for m in in_maps
        ]
        sharding = NamedSharding(mesh, PartitionSpec("core"))
        self._dev_in = [
            jax.device_put(
                np.concatenate([per_core[c][i] for c in range(N_CORES)], axis=0),
                sharding,
            )
            for i in range(n_params)
        ]
        self._zero_shapes = [
            ((N_CORES * z.shape[0],) + z.shape[1:], z.dtype) for z in zero_outs
        ]
        self._out_names = out_names
        self._out_avals = out_avals
        self._jax = jax

    def __call__(self):
        zeros = [np.zeros(s, d) for s, d in self._zero_shapes]
        outs = self._fn(*self._dev_in, *zeros)
        self._jax.block_until_ready(outs)
        return outs

    def results(self, outs):
        res = []
        for c in range(N_CORES):
            res.append(
                {
                    name: np.asarray(outs[i]).reshape(
                        N_CORES, *self._out_avals[i].shape
                    )[c]
                    for i, name in enumerate(self._out_names)
                }
            )
        return res


def kernel(**inputs):
    return _run(inputs)[0]
